# revision 10
# baseline (speedup 1.0000x reference)
"""Single-head attention (B=4, T=4096, E=1024, H=64) on 8 trn2 NeuronCores.

Sharding: 2 cores per batch element; each core computes the full K/V
projections for its batch element but only its half of the queries
(sequence-parallel over queries, data-parallel over batch). The host
permutes each core's token order so its own query half comes first —
attention is permutation-invariant over keys, so every core runs an
identical SPMD program with no collectives.

Per-core on-chip pipeline (all layouts transposed, feature-on-partition):
  xT [E,T] bf16 --matmul (Wk|Wv) packed--> K^T,V^T [64,T] f32
             --matmul Wq (first T/2 cols)--> Q^T [64,T/2]
  V^T --PE transpose--> V [T,64] stored as [128, T/128, 64|ones]
  S^T chunk = K^T_chunk.T @ Q^T  (contraction H=64, PSUM [128,512])
  P^T = exp(S^T/8)               (ScalarE, PSUM->SBUF, groups of 1024)
  O^T += [V|ones].T @ P^T        (PSUM [128,512]: rows 0:64 = O^T,
                                  rows 64:128 = softmax denominator)
  out = O^T * reciprocal(l)      (VectorE), DMA out [64, T/2] f32
"""

import os
import sys

import numpy as np

E, T, H, B = 1024, 4096, 64, 4
NCORES = 8
TQ = T // 2

_BUILT = {}
LAST_RESULT = None  # stashed BassKernelResults for test harness introspection


def _ensure_paths():
    for p in ("/opt/trn_rl_repo",):
        if p not in sys.path:
            sys.path.insert(0, p)


def _legalize_waits(nc, mybir, max_waits=1):
    """This walrus build only accepts 1 sem-wait per instruction; Tile's
    tail drains carry several. Move excess waits onto injected NoOps on
    the same engine right before the offending instruction."""
    ctr = 0
    for bb in nc.main_func.blocks:
        new_list = []
        for ins in bb.instructions:
            si = ins.sync_info
            if si is not None and len(si.on_wait) > max_waits:
                waits = list(si.on_wait)
                extra, keep = waits[:-max_waits], waits[-max_waits:]
                while extra:
                    chunk, extra = extra[:max_waits], extra[max_waits:]
                    ctr += 1
                    nop = mybir.InstNoOp(name=f"WFIX-{id(nc) & 0xFFFF}-{ctr}")
                    nop.engine = ins.engine
                    nop.sync_info = mybir.SyncInfo(on_wait=chunk, on_update=[])
                    new_list.append(nop)
                ins.sync_info = mybir.SyncInfo(
                    on_wait=keep, on_update=list(si.on_update)
                )
            new_list.append(ins)
        bb.instructions[:] = new_list


def _install_ntff_hook():
    """The image's antenv lacks axon_hooks, so trace=True degrades. Inject
    the module backed by the boot helper's ctypes implementation."""
    import types

    if "antenv.axon_hooks" in sys.modules:
        return
    if "/root/.axon_site" not in sys.path:
        sys.path.insert(0, "/root/.axon_site")
    try:
        from trn_agent_boot.trn_boot import _ntff_profile_via_ctypes

        hook = _ntff_profile_via_ctypes("/opt/axon/libaxon_pjrt.so")
    except Exception:
        return
    mod = types.ModuleType("antenv.axon_hooks")
    mod.get_axon_ntff_profile_hook = lambda: hook
    mod.set_axon_ntff_profile_hook = lambda h: None
    sys.modules["antenv.axon_hooks"] = mod


def build_nc(e=E, t=T, tq=TQ, rowtile=True, gfd=1024, legalize=True):
    """Emit the SPMD per-core program. Shapes parameterized so the same
    builder is validated in CoreSim at mini scale."""
    _ensure_paths()
    import concourse.bass as bass
    import concourse.mybir as mybir
    import concourse.tile as tile
    from concourse.masks import make_identity
    from contextlib import ExitStack

    f32 = mybir.dt.float32
    bf16 = mybir.dt.bfloat16
    fp8 = mybir.dt.float8e4
    DR = mybir.MatmulPerfMode.DoubleRow
    Exp = mybir.ActivationFunctionType.Exp

    EC = e // 128      # E (contraction) chunks for projections
    TT = t // 512      # token tiles (projection streaming)
    TTQ = tq // 512    # token tiles that also need Q projection
    KC = t // 128      # key chunks (attention contraction)
    QTN = tq // 512    # query tiles in attention
    GK = gfd // 512    # key chunks per exp group
    NG = KC // GK      # exp groups per query tile
    assert KC % GK == 0 and (not rowtile or GK == 2)

    nc = bass.Bass()
    xT = nc.declare_dram_parameter("xT", [e, t], bf16, False)
    wkv = nc.declare_dram_parameter("wkv", [e, 2 * H], bf16, False)
    wq = nc.declare_dram_parameter("wq", [e, H], bf16, False)
    outT = nc.declare_dram_parameter("outT", [H, tq], f32, True)

    xT_r = xT.rearrange("(c p) (n u) -> p c n u", p=128, u=512)
    wkv_r = wkv.rearrange("(c p) m -> p c m", p=128)
    wq_r = wq.rearrange("(c p) m -> p c m", p=128)

    with ExitStack() as ctx:
        tc = ctx.enter_context(tile.TileContext(nc))
        singles = ctx.enter_context(tc.tile_pool(name="singles", bufs=1))
        xpool = ctx.enter_context(tc.tile_pool(name="xpool", bufs=3))
        ppool = ctx.enter_context(tc.tile_pool(name="ppool", bufs=3))
        rpool = ctx.enter_context(tc.tile_pool(name="rpool", bufs=2))
        spool = ctx.enter_context(tc.tile_pool(name="spool", bufs=3, space="PSUM"))
        opool = ctx.enter_context(tc.tile_pool(name="opool", bufs=2, space="PSUM"))

        wkv_sb = singles.tile([128, EC, 2 * H], bf16)
        nc.sync.dma_start(out=wkv_sb, in_=wkv_r)
        wq_sb = singles.tile([128, EC, H], bf16)
        nc.sync.dma_start(out=wq_sb, in_=wq_r)
        ident = singles.tile([H, H], bf16)
        make_identity(nc, ident)

        # Per-token-tile storage so Tile's per-tile dependency tracking
        # lets attention start as soon as the first K/Q tiles land.
        # K^T rowtiled: [128, 256] per token tile -- even key chunks on
        # partitions 0:64, odd on 64:128, two 128-col blocks per tile.
        KTp = [
            singles.tile([128, 256] if rowtile else [H, 512], bf16, name=f"KT{n}")
            for n in range(TT)
        ]
        # Q^T per query tile, duplicated across both partition halves when
        # rowtiled (each concurrent row-tile streams rhs from its own range).
        QTp = [
            singles.tile([128, 512] if rowtile else [H, 512], bf16, name=f"QT{n}")
            for n in range(TTQ)
        ]
        VTtp = [singles.tile([H, 512], bf16, name=f"VTt{n}") for n in range(TT)]
        # PV stationary per key chunk: [V_chunk | ones], ones replicated to
        # fill M=128 so rows 64:128 of the PV accumulator hold the softmax
        # denominator.
        Vstp = [singles.tile([128, 128], bf16, name=f"Vst{c}") for c in range(KC)]
        for c in range(KC):
            nc.vector.memset(Vstp[c][:, H:], 1.0)
        OTp = [singles.tile([H, 512], f32, name=f"OT{q}") for q in range(QTN)]

        # ---- emission: projections interleaved with q0 attention ----
        # Emitting each projection tile, its V transposes, then the q0
        # attention groups that only need data produced so far lets the
        # Scalar engine start exp ~30us earlier instead of idling behind
        # the whole projection phase.
        scale = 1.0 / float(np.sqrt(H))
        o_ps_list = [None] * QTN

        def emit_group(q, g, o_ps):
            s_ps = spool.tile([128, gfd], f32, tag="s", name=f"s{q}_{g}")
            if rowtile:
                kt = KTp[g // 2][:, (g % 2) * 128:(g % 2 + 1) * 128]
                nc.tensor.matmul(
                    s_ps[:, 0:512], kt[0:H], QTp[q][0:H, :],
                    start=True, stop=True, skip_group_check=True,
                )
                nc.tensor.matmul(
                    s_ps[:, 512:1024], kt[H:128], QTp[q][H:128, :],
                    start=True, stop=True, skip_group_check=True,
                    tile_position=(64, 0),
                )
            else:
                for k in range(GK):
                    c = g * GK + k
                    nc.tensor.matmul(
                        s_ps[:, k * 512:(k + 1) * 512],
                        KTp[c // 4][:, (c % 4) * 128:(c % 4 + 1) * 128],
                        QTp[q][0:H, :],
                        start=True, stop=True, skip_group_check=True,
                    )
            pt = ppool.tile([128, gfd], bf16, tag="p", name=f"p{q}_{g}")
            nc.scalar.activation(pt, s_ps, Exp, scale=scale)
            for k in range(GK):
                c = g * GK + k
                nc.tensor.matmul(
                    o_ps, Vstp[c][:], pt[:, k * 512:(k + 1) * 512],
                    start=(c == 0), stop=(c == KC - 1),
                    skip_group_check=True,
                )

        def emit_finalize(q, o_ps):
            rec = rpool.tile([H, 512], f32, tag="rec", name=f"rec{q}")
            nc.vector.reciprocal(rec, o_ps[H:128, :])
            nc.vector.tensor_mul(OTp[q][:], o_ps[0:H, :], rec)
            nc.sync.dma_start(
                out=outT[:, q * 512:(q + 1) * 512], in_=OTp[q][:]
            )

        for n in range(TT):
            # x tile as per-chunk tiles so the first matmul starts after the
            # first 128KB lands rather than the full 1MB.
            xtc = [
                xpool.tile([128, 512], bf16, tag=f"x{c}", name=f"x{n}_{c}")
                for c in range(EC)
            ]
            for c in range(EC):
                nc.sync.dma_start(out=xtc[c], in_=xT_r[:, c, n, :])
            kv_ps = spool.tile([128, 512], f32, tag="s", name=f"kv{n}")
            for c in range(EC):
                nc.tensor.matmul(
                    kv_ps, wkv_sb[:, c, :], xtc[c],
                    start=(c == 0), stop=(c == EC - 1),
                )
            if rowtile:
                srcv = kv_ps[0:H, :].rearrange("h (i r u) -> h i r u", i=2, r=2, u=128)
                dst = KTp[n].rearrange("p (i u) -> p i u", u=128)
                nc.vector.tensor_copy(dst[0:H], srcv[:, :, 0, :])
                nc.vector.tensor_copy(dst[H:128], srcv[:, :, 1, :])
            else:
                nc.vector.tensor_copy(KTp[n][:], kv_ps[0:H, :])
            nc.vector.tensor_copy(VTtp[n][:], kv_ps[H:128, :])
            if n < TTQ:
                q_ps = spool.tile([H, 512], f32, tag="s", name=f"q{n}")
                for c in range(EC):
                    nc.tensor.matmul(
                        q_ps, wq_sb[:, c, :], xtc[c],
                        start=(c == 0), stop=(c == EC - 1),
                    )
                nc.vector.tensor_copy(QTp[n][0:H, :], q_ps)
                if rowtile:
                    nc.vector.tensor_copy(QTp[n][H:128, :], q_ps)
            for c in range(4 * n, 4 * n + 4):
                if c < KC:
                    tp = opool.tile([128, H], bf16, tag="o", name=f"tp{c}")
                    nc.tensor.transpose(
                        tp, VTtp[c // 4][:, (c % 4) * 128:(c % 4 + 1) * 128], ident
                    )
                    nc.vector.tensor_copy(Vstp[c][:, 0:H], tp)
            # q0 attention groups feasible with tiles <= n
            if o_ps_list[0] is None:
                o_ps_list[0] = opool.tile([128, 512], f32, tag="o", name="o0")
            for g in range(2 * n, min(2 * n + 2, NG)):
                emit_group(0, g, o_ps_list[0])

        for g in range(2 * TT, NG):
            emit_group(0, g, o_ps_list[0])
        emit_finalize(0, o_ps_list[0])
        for q in range(1, QTN):
            o_ps_list[q] = opool.tile([128, 512], f32, tag="o", name=f"o{q}")
            for g in range(NG):
                emit_group(q, g, o_ps_list[q])
            emit_finalize(q, o_ps_list[q])

    if legalize:
        _legalize_waits(nc, __import__("concourse.mybir", fromlist=["x"]))
    return nc


def _get_nc():
    key = (E, T, TQ)
    if key not in _BUILT:
        _BUILT[key] = build_nc()
    return _BUILT[key]


def kernel(x, Wq, Wk, Wv):
    """Full inputs -> full output, distributing over 8 NeuronCores."""
    _ensure_paths()
    _install_ntff_hook()
    import ml_dtypes
    from concourse.bass_utils import run_bass_kernel_spmd

    global LAST_RESULT

    nc = _get_nc()

    x = np.asarray(x, np.float32)
    wkv_np = np.ascontiguousarray(
        np.concatenate([np.asarray(Wk, np.float32), np.asarray(Wv, np.float32)], axis=1)
    ).astype(ml_dtypes.bfloat16)
    wq_np = np.ascontiguousarray(np.asarray(Wq, np.float32)).astype(ml_dtypes.bfloat16)

    in_maps = []
    for core in range(NCORES):
        b, half = divmod(core, 2)
        o = TQ if half == 0 else 0
        idx = np.r_[half * TQ:(half + 1) * TQ, o:o + TQ]
        xT_perm = np.ascontiguousarray(x[b, idx].T).astype(ml_dtypes.bfloat16)
        in_maps.append({"xT": xT_perm, "wkv": wkv_np, "wq": wq_np})

    trace = bool(os.environ.get("KERNEL_TRACE"))
    res = run_bass_kernel_spmd(nc, in_maps, list(range(NCORES)), trace=trace)
    LAST_RESULT = res

    out = np.empty((B, T, H), np.float32)
    for core in range(NCORES):
        b, half = divmod(core, 2)
        out[b, half * TQ:(half + 1) * TQ, :] = res.results[core]["outT"].T
    return out
